# revision 9
# baseline (speedup 1.0000x reference)
"""1x1 conv (channel reduction) kernel for Trainium2.

out[s, a] = sum_c w[c] * x[s, c, a] + b
x: (64, 1024, 4096) f32, w: (1024,) f32, b: () f32 -> out: (64, 4096) f32

Sharding: data-parallel over samples; 8 samples per core on 8 cores.

Per core, the channel (partition axis) reduction runs on the TensorEngine.
A plain fp32 matmul costs 4 PE cycles/row, which makes the PE the
bottleneck (~440us/core vs the ~375us HBM roofline). Instead x is split
on the fly into fp16 hi+lo halves (exact to 22 mantissa bits):
  x = xh + xl            xh = fp16(x) (ScalarE cast), xl = fp16(x - xh) (VectorE)
  w = wh + dw            wh = fp16(w), dws = fp16(dw * 2^13)  (host precomputed)
  out = wh*xh + wh*xl + (dws*xh) * 2^-13 + b     (3 fp16 matmuls = 3 PE cyc/row)
The dropped dw*xl term is ~2^-23 relative. Main accumulates in PSUM at
partition base 0/64 (alternating per sample), the scaled correction at
base 32; they are merged during the PSUM->SBUF eviction.
"""

import contextlib
import ctypes
import sys
import types

import numpy as np

import concourse.bacc as bacc
import concourse.bass as bass
import concourse.mybir as mybir
import concourse.tile as tile
from concourse import bass_utils


def _ensure_ntff_hook():
    """bass_utils.run_bass_kernel_spmd(trace=True) under axon needs
    antenv.axon_hooks, which this image's antenv lacks. Provide it and
    register the ctypes NTFF hook against the axon PJRT .so."""
    try:
        import antenv.axon_hooks  # noqa: F401
        return
    except ImportError:
        pass
    mod = types.ModuleType("antenv.axon_hooks")
    state = {"hook": None}
    mod.set_axon_ntff_profile_hook = lambda h: state.__setitem__("hook", h)
    mod.get_axon_ntff_profile_hook = lambda: state["hook"]
    sys.modules["antenv.axon_hooks"] = mod
    try:
        import antenv
        antenv.axon_hooks = mod
    except ImportError:
        pass

    so_path = "/opt/axon/libaxon_pjrt.so"
    try:
        lib = ctypes.CDLL(so_path)
    except OSError:
        return
    if not hasattr(lib, "axon_start_nrt_profile"):
        return
    lib.axon_start_nrt_profile.argtypes = [
        ctypes.POINTER(ctypes.c_int64),
        ctypes.c_size_t,
    ]
    lib.axon_start_nrt_profile.restype = ctypes.c_int64
    lib.axon_stop_nrt_profile.argtypes = [ctypes.c_char_p]
    lib.axon_stop_nrt_profile.restype = ctypes.c_int64

    @contextlib.contextmanager
    def _hook(output_dir, device_ids):
        import jax

        jax.devices()
        if device_ids:
            ids = (ctypes.c_int64 * len(device_ids))(*device_ids)
            rc = lib.axon_start_nrt_profile(ids, len(device_ids))
        else:
            rc = lib.axon_start_nrt_profile(None, 0)
        if rc != 0:
            raise RuntimeError(f"axon_start_nrt_profile rc={rc}")
        try:
            yield
        finally:
            n = lib.axon_stop_nrt_profile(str(output_dir).encode())
            print(f"ntff profile: {n} file(s) written to {output_dir}",
                  file=sys.stderr)

    mod.set_axon_ntff_profile_hook(_hook)


_ensure_ntff_hook()

N_CORES = 8
S, C, A = 64, 1024, 4096
SP = S // N_CORES  # samples per core
P = 128  # partitions / channel-chunk size
CHUNKS = C // P  # 8
F = 512  # matmul moving free dim (one PSUM bank of f32)
NF = A // F  # 8
CORR_SCALE = 2.0 ** 13

_cache: dict = {}


def _build_fp16split():
    nc = bacc.Bacc("TRN2", target_bir_lowering=False, debug=False)
    f32 = mybir.dt.float32
    f16 = mybir.dt.float16

    x_d = nc.dram_tensor("x", (SP, C, A), f32, kind="ExternalInput")
    wh_d = nc.dram_tensor("wh", (C,), f16, kind="ExternalInput")
    dws_d = nc.dram_tensor("dws", (C,), f16, kind="ExternalInput")
    b_d = nc.dram_tensor("b", (1, 1), f32, kind="ExternalInput")
    o_d = nc.dram_tensor("out", (SP, A), f32, kind="ExternalOutput")

    with tile.TileContext(nc) as tc:
        with (
            tc.tile_pool(name="const", bufs=1) as cpool,
            tc.tile_pool(name="xs", bufs=3) as xpool,
            tc.tile_pool(name="xh", bufs=4) as hpool,
            tc.tile_pool(name="xl", bufs=4) as lpool,
            tc.tile_pool(name="ps", bufs=1, space=bass.MemorySpace.PSUM) as ppool,
            tc.tile_pool(name="os", bufs=2) as opool,
        ):
            # weight columns: wh_t[p, k] = wh[128k + p]
            wh_t = cpool.tile([P, CHUNKS], f16)
            nc.sync.dma_start(wh_t[:], wh_d.ap().rearrange("(k p) -> p k", p=P))
            dws_t = cpool.tile([P, CHUNKS], f16)
            nc.sync.dma_start(dws_t[:], dws_d.ap().rearrange("(k p) -> p k", p=P))
            # bias replicated at partition 32 (matches corr psum base)
            b_t = cpool.tile([33, 1], f32)
            nc.sync.dma_start(b_t[32:33, :], b_d.ap())

            # one psum tile: main rows at partitions {0, 64} (alternating by
            # sample), scaled correction row at partition 32
            psum_t = ppool.tile([65, A], f32)
            xv = x_d.ap()
            for s in range(SP):
                mb = 0 if s % 2 == 0 else 64  # main psum base partition
                main = psum_t[mb : mb + 1, :]
                corr = psum_t[32:33, :]
                for k in range(CHUNKS):
                    xt = xpool.tile([P, A], f32)
                    nc.sync.dma_start(xt[:], xv[s, P * k : P * (k + 1), :])
                    xh_t = hpool.tile([P, A], f16)
                    xl_t = lpool.tile([P, A], f16)
                    # split the cast/sub elementwise work across engines so
                    # none exceeds the DMA floor: even chunks ACT+DVE, odd
                    # chunks DVE+GpSimd
                    if k % 2 == 0:
                        nc.scalar.copy(xh_t[:], xt[:])
                        nc.vector.tensor_tensor(
                            xl_t[:], xt[:], xh_t[:], op=mybir.AluOpType.subtract
                        )
                    else:
                        nc.vector.tensor_copy(xh_t[:], xt[:])
                        nc.gpsimd.tensor_tensor(
                            xl_t[:], xt[:], xh_t[:], op=mybir.AluOpType.subtract
                        )
                    for j in range(NF):
                        js = slice(F * j, F * (j + 1))
                        nc.tensor.matmul(
                            main[:, js], wh_t[:, k : k + 1], xh_t[:, js],
                            start=(k == 0), stop=False,
                        )
                        nc.tensor.matmul(
                            main[:, js], wh_t[:, k : k + 1], xl_t[:, js],
                            start=False, stop=(k == CHUNKS - 1),
                        )
                        nc.tensor.matmul(
                            corr[:, js], dws_t[:, k : k + 1], xh_t[:, js],
                            start=(k == 0), stop=(k == CHUNKS - 1),
                        )

                # eviction: out = main + (corr * 2^-13 + b); the corr read is
                # one ACT Identity op (scale+bias), the final add is on DVE
                corr_sb = opool.tile([1, A], f32, tag="corr_sb")
                nc.scalar.activation(
                    corr_sb[:], corr, mybir.ActivationFunctionType.Identity,
                    bias=b_t[32:33, :], scale=1.0 / CORR_SCALE,
                )
                out_sb = opool.tile([1, A], f32, tag="out_sb")
                nc.vector.tensor_tensor(
                    out_sb[:], main, corr_sb[:], op=mybir.AluOpType.add
                )
                nc.sync.dma_start(o_d.ap()[s : s + 1, :], out_sb[:])

    nc.compile()
    return nc


def _build_fp32():
    """Reference implementation: plain fp32 matmuls (4 PE cyc/row)."""
    nc = bacc.Bacc("TRN2", target_bir_lowering=False, debug=False)
    f32 = mybir.dt.float32

    x_d = nc.dram_tensor("x", (SP, C, A), f32, kind="ExternalInput")
    w_d = nc.dram_tensor("w", (C,), f32, kind="ExternalInput")
    b_d = nc.dram_tensor("b", (1, 1), f32, kind="ExternalInput")
    o_d = nc.dram_tensor("out", (SP, A), f32, kind="ExternalOutput")

    with tile.TileContext(nc) as tc:
        with (
            tc.tile_pool(name="const", bufs=1) as cpool,
            tc.tile_pool(name="xs", bufs=4) as xpool,
            tc.tile_pool(name="ps", bufs=1, space=bass.MemorySpace.PSUM) as ppool,
            tc.tile_pool(name="os", bufs=2) as opool,
        ):
            w_t = cpool.tile([P, CHUNKS], f32)
            nc.sync.dma_start(w_t[:], w_d.ap().rearrange("(k p) -> p k", p=P))
            b_t = cpool.tile([1, 1], f32)
            nc.sync.dma_start(b_t[:], b_d.ap())

            xv = x_d.ap()
            for s in range(SP):
                psum_t = ppool.tile([1, A], f32)
                for k in range(CHUNKS):
                    xt = xpool.tile([P, A], f32)
                    nc.sync.dma_start(xt[:], xv[s, P * k : P * (k + 1), :])
                    for j in range(NF):
                        nc.tensor.matmul(
                            psum_t[:, F * j : F * (j + 1)],
                            w_t[:, k : k + 1],
                            xt[:, F * j : F * (j + 1)],
                            start=(k == 0),
                            stop=(k == CHUNKS - 1),
                        )

                o_t = opool.tile([1, A], f32)
                nc.vector.tensor_scalar_add(o_t[:], psum_t[:], b_t[:])
                nc.sync.dma_start(o_d.ap()[s : s + 1, :], o_t[:])

    nc.compile()
    return nc


def _get_nc(mode: str = "fp16split"):
    key = ("nc", mode)
    if key not in _cache:
        _cache[key] = {
            "fp16split": _build_fp16split,
            "fp32": _build_fp32,
        }[mode]()
    return _cache[key]


def kernel(x: np.ndarray, w: np.ndarray, b: np.ndarray, trace: bool = False,
           mode: str = "fp16split"):
    x = np.ascontiguousarray(np.asarray(x, dtype=np.float32))
    w = np.ascontiguousarray(np.asarray(w, dtype=np.float32))
    b_arr = np.asarray(b, dtype=np.float32).reshape(1, 1)

    nc = _get_nc(mode)
    if mode == "fp16split":
        wh = w.astype(np.float16)
        dws = ((w - wh.astype(np.float32)) * CORR_SCALE).astype(np.float16)
        in_maps = [
            {"x": x[i * SP : (i + 1) * SP], "wh": wh, "dws": dws, "b": b_arr}
            for i in range(N_CORES)
        ]
    else:
        in_maps = [
            {"x": x[i * SP : (i + 1) * SP], "w": w, "b": b_arr}
            for i in range(N_CORES)
        ]
    res = bass_utils.run_bass_kernel_spmd(
        nc, in_maps, core_ids=list(range(N_CORES)), trace=trace
    )
    out = np.concatenate([r["out"] for r in res.results], axis=0)
    if trace:
        kernel.last_exec_time_ns = res.exec_time_ns
        kernel.last_results = res
    return out
